# revision 6
# baseline (speedup 1.0000x reference)
"""GQA with sliding-window + ALiBi (reduces to banded causal attention) on 8 TRN2 cores.

Sharding: 8 cores = 2 batches x 4 kv-head groups. Each core computes, for its
(batch b, kv group gi): Q projection for its 4 query heads, K/V projection for
its 1 kv head, banded sliding-window attention (window 1024, causal), and a
partial row-parallel Wo matmul. Host sums the 4 partials per batch.

Math notes (exact reductions of the reference):
- ALiBi bias is -clip(j-i,0)*slope: zero on all causal positions, nonzero only
  where the causal mask kills the score -> drop it entirely.
- The sliding mask adds +1.0 uniformly inside the window: softmax-invariant.
- Out-of-window/causal positions get -1e9 -> exp underflows to exactly 0.
- Scores are O(1), so softmax without max-subtraction is safe in fp32.
All matmuls run as float32r (measured bit-identical to fp32 on TRN2 HW, 4x rate).
"""
import math
from contextlib import ExitStack

import numpy as np

import concourse.tile as tile
from concourse import bacc, mybir
from concourse.bass_utils import run_bass_kernel_spmd
from concourse.masks import make_identity

dt = mybir.dt

B, S, H = 2, 2048, 2048
NUM_HEADS, KV_HEADS, D = 16, 4, 128
WINDOW = 1024
GH = 4            # query heads per kv head (per core)
GD = GH * D       # 512: per-core slice of the hidden dim
SCALE = 1.0 / math.sqrt(D)
NEG = -1e9
QB = 256          # query columns per attention group (2 blocks of 128)
NG = S // QB      # 8 query groups
KT = H // 128     # 16 contraction tiles for projections

_nc_cache = None


def _build_nc():
    nc = bacc.Bacc()
    hsT = nc.declare_dram_parameter("hsT", [H, S], dt.float32r, isOutput=False)
    wq = nc.declare_dram_parameter("wq", [H, GD], dt.float32r, isOutput=False)
    wk = nc.declare_dram_parameter("wk", [H, D], dt.float32r, isOutput=False)
    wv = nc.declare_dram_parameter("wv", [H, D], dt.float32r, isOutput=False)
    wo = nc.declare_dram_parameter("wo", [GD, H], dt.float32r, isOutput=False)
    masks = nc.declare_dram_parameter("masks", [4, 128, QB], dt.float32, isOutput=False)
    out = nc.declare_dram_parameter("out", [S, H], dt.float32, isOutput=True)

    with tile.TileContext(nc) as tc, ExitStack() as ctx:
        consts = ctx.enter_context(tc.tile_pool(name="consts", bufs=1))
        wpool = ctx.enter_context(tc.tile_pool(name="wpool", bufs=1))
        big = ctx.enter_context(tc.tile_pool(name="big", bufs=1))
        hstp = ctx.enter_context(tc.tile_pool(name="hstp", bufs=2))
        vtp = ctx.enter_context(tc.tile_pool(name="vtp", bufs=2))
        ptp = ctx.enter_context(tc.tile_pool(name="ptp", bufs=2))
        smalls = ctx.enter_context(tc.tile_pool(name="smalls", bufs=4))
        outp = ctx.enter_context(tc.tile_pool(name="outp", bufs=1))
        psum = ctx.enter_context(tc.tile_pool(name="psum", bufs=8, space="PSUM"))

        # constants
        ident32 = consts.tile([128, 128], dt.float32)
        make_identity(nc, ident32)
        ident = consts.tile([128, 128], dt.float32r)
        nc.vector.tensor_copy(ident, ident32)
        ones32 = consts.tile([128, 128], dt.float32)
        nc.vector.memset(ones32, 1.0)
        ones = consts.tile([128, 128], dt.float32r)
        nc.vector.tensor_copy(ones, ones32)
        mask_t = []
        for i in range(4):
            mt = consts.tile([128, QB], dt.float32, tag=f"mask{i}")
            nc.sync.dma_start(out=mt, in_=masks[i])
            mask_t.append(mt)

        # weights
        wq_t, wk_t, wv_t = [], [], []
        for t in range(KT):
            wqt = wpool.tile([128, GD], dt.float32r, tag=f"wq{t}")
            nc.sync.dma_start(out=wqt, in_=wq[t * 128:(t + 1) * 128, :])
            wq_t.append(wqt)
            wkt = wpool.tile([128, D], dt.float32r, tag=f"wk{t}")
            nc.sync.dma_start(out=wkt, in_=wk[t * 128:(t + 1) * 128, :])
            wk_t.append(wkt)
            wvt = wpool.tile([128, D], dt.float32r, tag=f"wv{t}")
            nc.sync.dma_start(out=wvt, in_=wv[t * 128:(t + 1) * 128, :])
            wv_t.append(wvt)
        wo_t = []
        for ct in range(4):
            wot = wpool.tile([128, H], dt.float32r, tag=f"wo{ct}")
            nc.sync.dma_start(out=wot, in_=wo[ct * 128:(ct + 1) * 128, :])
            wo_t.append(wot)

        # persistent activations
        qT = [big.tile([128, S], dt.float32r, tag=f"qT{h}", name=f"qT{h}") for h in range(GH)]
        kT = big.tile([128, S], dt.float32r, tag="kT")
        v = big.tile([128, S], dt.float32r, tag="v")
        ohT = [big.tile([128, S], dt.float32r, tag=f"ohT{h}", name=f"ohT{h}") for h in range(GH)]

        # ---- Phase 1: projections (per 512-wide s-chunk) ----
        for ch in range(4):
            q_ps = [psum.tile([128, 512], dt.float32, tag="ps", name=f"qps{ch}_{h}") for h in range(GH)]
            k_ps = psum.tile([128, 512], dt.float32, tag="ps")
            v_ps = psum.tile([128, 512], dt.float32, tag="ps")
            for t in range(KT):
                hst = hstp.tile([128, 512], dt.float32r, tag="hst")
                nc.sync.dma_start(
                    out=hst, in_=hsT[t * 128:(t + 1) * 128, ch * 512:(ch + 1) * 512])
                st = (t == 0)
                sp = (t == KT - 1)
                for h in range(GH):
                    nc.tensor.matmul(q_ps[h], lhsT=wq_t[t][:, h * 128:(h + 1) * 128],
                                     rhs=hst, start=st, stop=sp)
                nc.tensor.matmul(k_ps, lhsT=wk_t[t], rhs=hst, start=st, stop=sp)
                nc.tensor.matmul(v_ps, lhsT=wv_t[t], rhs=hst, start=st, stop=sp)
            for h in range(GH):
                nc.vector.tensor_copy(qT[h][:, ch * 512:(ch + 1) * 512], q_ps[h])
            nc.vector.tensor_copy(kT[:, ch * 512:(ch + 1) * 512], k_ps)
            vt = vtp.tile([128, 512], dt.float32r, tag="vt")
            nc.vector.tensor_copy(vt, v_ps)
            for j in range(4):
                tp = psum.tile([128, 128], dt.float32r, tag="ps")
                nc.tensor.transpose(tp, vt[:, j * 128:(j + 1) * 128], ident)
                nc.vector.tensor_copy(
                    v[:, (4 * ch + j) * 128:(4 * ch + j + 1) * 128], tp)

        # ---- Phase 2: banded attention, scores transposed (S^T[k, q]) ----
        mask_for_o = {1: 1, 0: 0, -7: 3, -8: 2}
        for h in range(GH):
            for g in range(NG):
                kjs = list(range(max(0, 2 * g - 8), 2 * g + 2))
                av = psum.tile([128, QB], dt.float32, tag="ps")
                den = psum.tile([1, QB], dt.float32, tag="ps")
                batches = [kjs[i:i + 2] for i in range(0, len(kjs), 2)]
                for bi, bk in enumerate(batches):
                    sps = psum.tile([128, QB * len(bk)], dt.float32, tag="ps")
                    for idx, kj in enumerate(bk):
                        nc.tensor.matmul(
                            sps[:, idx * QB:(idx + 1) * QB],
                            lhsT=kT[:, kj * 128:(kj + 1) * 128],
                            rhs=qT[h][:, g * QB:(g + 1) * QB],
                            start=True, stop=True)
                        mi = mask_for_o.get(kj - 2 * g)
                        if mi is not None:
                            nc.vector.tensor_add(
                                sps[:, idx * QB:(idx + 1) * QB],
                                sps[:, idx * QB:(idx + 1) * QB], mask_t[mi])
                    pt = ptp.tile([128, QB * 2], dt.float32r, tag="pt")
                    nc.scalar.activation(
                        pt[:, :QB * len(bk)], sps,
                        mybir.ActivationFunctionType.Exp, scale=SCALE)
                    for idx, kj in enumerate(bk):
                        first = (bi == 0 and idx == 0)
                        last = (kj == kjs[-1])
                        nc.tensor.matmul(
                            den, lhsT=ones[:, 0:1],
                            rhs=pt[:, idx * QB:(idx + 1) * QB],
                            start=first, stop=last)
                        nc.tensor.matmul(
                            av, lhsT=v[:, kj * 128:(kj + 1) * 128],
                            rhs=pt[:, idx * QB:(idx + 1) * QB],
                            start=first, stop=last)
                rc = smalls.tile([1, QB], dt.float32r, tag="rc")
                with nc.allow_low_precision(reason="f32r is full fp32 bits"):
                    nc.vector.reciprocal(rc, den)
                bc = psum.tile([128, QB], dt.float32, tag="ps")
                nc.tensor.matmul(bc, lhsT=ones[0:1, :], rhs=rc, start=True, stop=True)
                bcs = smalls.tile([128, QB], dt.float32, tag="bcs")
                nc.vector.tensor_copy(bcs, bc)
                nc.vector.tensor_mul(ohT[h][:, g * QB:(g + 1) * QB], av, bcs)

        # ---- Phase 3: partial Wo (row-parallel) ----
        for st in range(16):
            osb = outp.tile([128, H], dt.float32, tag="osb")
            for e in range(4):
                wops = psum.tile([128, 512], dt.float32, tag="ps")
                for ct in range(4):
                    nc.tensor.matmul(
                        wops, lhsT=ohT[ct][:, st * 128:(st + 1) * 128],
                        rhs=wo_t[ct][:, e * 512:(e + 1) * 512],
                        start=(ct == 0), stop=(ct == 3))
                nc.vector.tensor_copy(osb[:, e * 512:(e + 1) * 512], wops)
            nc.sync.dma_start(out=out[st * 128:(st + 1) * 128, :], in_=osb)

    nc.compile()
    return nc


def _build_masks():
    kk = np.arange(128)[:, None]
    qq = np.arange(128)[None, :]
    diag = np.where(kk <= qq, 0.0, NEG).astype(np.float32)
    edge = np.where(kk >= qq, 0.0, NEG).astype(np.float32)
    full = np.full((128, 128), NEG, np.float32)
    none = np.zeros((128, 128), np.float32)
    return np.stack([
        np.hstack([diag, none]),   # o = 0
        np.hstack([full, diag]),   # o = +1
        np.hstack([edge, full]),   # o = -8
        np.hstack([none, edge]),   # o = -7
    ])


def kernel(hidden_states, Wq, Wk, Wv, Wo):
    global _nc_cache
    if _nc_cache is None:
        _nc_cache = _build_nc()
    nc = _nc_cache

    masks = _build_masks()
    hsT = [np.ascontiguousarray(hidden_states[b].T) for b in range(B)]
    in_maps = []
    for b in range(B):
        for gi in range(KV_HEADS):
            in_maps.append({
                "hsT": hsT[b],
                "wq": np.ascontiguousarray(Wq[:, gi * GD:(gi + 1) * GD]),
                "wk": np.ascontiguousarray(Wk[:, gi * D:(gi + 1) * D]),
                "wv": np.ascontiguousarray(Wv[:, gi * D:(gi + 1) * D]),
                "wo": np.ascontiguousarray(Wo[gi * GD:(gi + 1) * GD, :]),
                "masks": masks,
            })
    res = run_bass_kernel_spmd(nc, in_maps, list(range(8)))
    out = np.zeros((B, S, H), np.float32)
    for b in range(B):
        for gi in range(KV_HEADS):
            out[b] += res.results[b * KV_HEADS + gi]["out"]
    return out
